# revision 3
# baseline (speedup 1.0000x reference)
"""Grouped (depthwise-multiplier-2) 3x3 conv + bias + BatchNorm(inference) + SiLU
on 8 Trainium2 NeuronCores.

Problem (hardcoded): x [16, 256, 80, 80] f32, w [512, 1, 3, 3] (256 groups,
2 out-channels per group), b/bn_* [512]. Output [16, 512, 80, 80] f32.

Strategy:
- Data-parallel over batch: 2 images per core, same NEFF on all 8 cores (SPMD).
- BN is folded into conv weights/bias on the host:
    inv = gamma / sqrt(var + eps);  w' = w * inv[o];  b' = b*inv + beta - mean*inv
- Depthwise conv runs on the TensorEngine as fp32r matmuls (TF32-like, 12-bit
  mantissa, 1 col/cycle for N>=256) with a 2-phase layout: one half of the
  SBUF partitions holds a 64-channel block of the (host-padded 82x82) image,
  the other half the same channels shifted down one row. Block-diagonal lhsT
  matrices accumulate all 9 taps into PSUM over a 5-output-row chunk
  ([K] x [N=400]):
    MM_A(kw): K=128, phase0 rows -> kh=0 weights, phase1 rows -> kh=1
    MM_B(kw): K=64, phase1 rows @ +1 row offset -> kh=2 weights
- Channel blocks are processed in pairs with MIRRORED phase layouts (even
  block: phase0 on partitions 0..63; odd block: phase0 on 64..127) so the two
  half-height MM_Bs sit on disjoint PE row-groups (tile rows 64.. vs 0..)
  and execute concurrently in the systolic array.
- Output channels of block cb land on PSUM partitions as channels
  [128*cb, 128*cb+128) in order (o = 2c+parity is consecutive).
- Epilogue: one ScalarEngine pass Silu(psum + b') PSUM->SBUF; stores batched
  4 chunks (20 rows) per HWDGE DMA.
"""

import numpy as np

B, CIN, H, W = 16, 256, 80, 80
COUT = 512
KH = KW = 3
NCORES = 8
BPC = B // NCORES          # batches per core
HP = WP = H + 2            # host-padded image
NR = 5                     # output rows per PSUM chunk
NCHUNK = H // NR           # 16
N = NR * W                 # 400 (>=256 keeps fp32r at full speed)
CBLK = 64                  # input channels per block
NBLK = CIN // CBLK         # 4
GROUP = 4                  # chunks per output store
BN_EPS = 1e-5
NT = 4                     # input-tile ring (2 live + 2 prefetch)

_state = {}


def _build_module(nrep=1, loop_n=0):
    """nrep > 1 statically repeats the whole compute (idempotent output
    writes); loop_n > 0 additionally wraps it in a For_i hardware loop --
    both are used by the test harness to measure per-iteration HW time
    with launch overhead amortized."""
    import contextlib

    import concourse.tile as tile
    from concourse import bacc, mybir

    f32 = mybir.dt.float32
    f32r = mybir.dt.float32r
    silu = mybir.ActivationFunctionType.Silu

    nc = bacc.Bacc("TRN2", target_bir_lowering=False, debug=False, num_devices=NCORES)
    xp_d = nc.dram_tensor("xp", [BPC, CIN, HP * WP], f32, kind="ExternalInput").ap()
    w_d = nc.dram_tensor(
        "wpack", [128, NBLK * 6 * 128], f32, kind="ExternalInput"
    ).ap()
    b_d = nc.dram_tensor("bpack", [128, NBLK], f32, kind="ExternalInput").ap()
    y_d = nc.dram_tensor("y", [BPC, COUT, H * W], f32, kind="ExternalOutput").ap()

    def widx(cb, kw, ab):
        return ((cb * 3 + kw) * 2 + ab) * 128

    with tile.TileContext(nc) as tc:
        with (
            tc.tile_pool(name="const", bufs=1) as cpool,
            tc.tile_pool(name="tin", bufs=1) as tpool,
            tc.tile_pool(name="outp", bufs=4) as outp,
            tc.tile_pool(name="psum", bufs=3, space="PSUM") as pspool,
        ):
            wt_f = cpool.tile([128, NBLK * 6 * 128], f32, tag="wt_f")
            nc.sync.dma_start(wt_f[:], w_d)
            wt = cpool.tile([128, NBLK * 6 * 128], f32r, tag="wt")
            nc.vector.tensor_copy(wt[:], wt_f[:])
            bt = cpool.tile([128, NBLK], f32, tag="bt")
            nc.sync.dma_start(bt[:], b_d)

            Ts = [
                tpool.tile([128, HP * WP], f32r, tag=f"T{i}", name=f"T{i}")
                for i in range(NT)
            ]

            loop_cm = (
                tc.For_i(0, loop_n, 1) if loop_n > 0 else contextlib.nullcontext()
            )
            it = 0
            with loop_cm:
                for _rep in range(nrep):
                    for bi in range(BPC):
                        for pr in range(NBLK // 2):
                            cb_e, cb_o = 2 * pr, 2 * pr + 1
                            T_e = Ts[it % NT]
                            T_o = Ts[(it + 1) % NT]
                            it += 2
                            # even block: phase0 on partitions 0..63
                            nc.gpsimd.dma_start(
                                T_e[0:CBLK, :],
                                xp_d[bi, cb_e * CBLK:(cb_e + 1) * CBLK, :],
                            )
                            nc.vector.tensor_copy(
                                T_e[CBLK:128, 0:(HP - 1) * WP],
                                T_e[0:CBLK, WP:HP * WP],
                            )
                            # odd block: phase0 on partitions 64..127
                            nc.gpsimd.dma_start(
                                T_o[CBLK:128, :],
                                xp_d[bi, cb_o * CBLK:(cb_o + 1) * CBLK, :],
                            )
                            nc.vector.tensor_copy(
                                T_o[0:CBLK, 0:(HP - 1) * WP],
                                T_o[CBLK:128, WP:HP * WP],
                            )
                            Tv_e = T_e[:].rearrange("p (r c) -> p r c", r=HP)
                            Tv_o = T_o[:].rearrange("p (r c) -> p r c", r=HP)

                            for g in range(NCHUNK // GROUP):
                                ot_e = outp.tile([128, GROUP * N], f32, tag="ot_e")
                                ot_o = outp.tile([128, GROUP * N], f32, tag="ot_o")
                                for ci in range(GROUP):
                                    r0 = (g * GROUP + ci) * NR
                                    ps_e = pspool.tile([128, N], f32, tag="ps_e")
                                    ps_o = pspool.tile([128, N], f32, tag="ps_o")
                                    # full-K taps (kh0 on phase0, kh1 on phase1)
                                    for kw in range(KW):
                                        i0 = widx(cb_e, kw, 0)
                                        nc.tensor.matmul(
                                            ps_e[:],
                                            wt[:, i0:i0 + 128],
                                            Tv_e[:, r0:r0 + NR, kw:kw + W],
                                            start=(kw == 0),
                                            stop=False,
                                        )
                                    for kw in range(KW):
                                        i0 = widx(cb_o, kw, 0)
                                        nc.tensor.matmul(
                                            ps_o[:],
                                            wt[:, i0:i0 + 128],
                                            Tv_o[:, r0:r0 + NR, kw:kw + W],
                                            start=(kw == 0),
                                            stop=False,
                                        )
                                    # kh=2 half-K taps, mirrored row-groups ->
                                    # concurrent in the PE array
                                    for kw in range(KW):
                                        i1e = widx(cb_e, kw, 1)
                                        i1o = widx(cb_o, kw, 1)
                                        nc.tensor.matmul(
                                            ps_e[:],
                                            wt[CBLK:128, i1e:i1e + 128],
                                            Tv_e[CBLK:128, r0 + 1:r0 + 1 + NR, kw:kw + W],
                                            start=False,
                                            stop=(kw == KW - 1),
                                        )
                                        nc.tensor.matmul(
                                            ps_o[:],
                                            wt[0:CBLK, i1o:i1o + 128],
                                            Tv_o[0:CBLK, r0 + 1:r0 + 1 + NR, kw:kw + W],
                                            start=False,
                                            stop=(kw == KW - 1),
                                        )
                                    nc.scalar.activation(
                                        ot_e[:, ci * N:(ci + 1) * N], ps_e[:],
                                        silu, bias=bt[:, cb_e:cb_e + 1], scale=1.0,
                                    )
                                    nc.scalar.activation(
                                        ot_o[:, ci * N:(ci + 1) * N], ps_o[:],
                                        silu, bias=bt[:, cb_o:cb_o + 1], scale=1.0,
                                    )
                                for cb, ot in ((cb_e, ot_e), (cb_o, ot_o)):
                                    nc.sync.dma_start(
                                        y_d[
                                            bi,
                                            cb * 128:(cb + 1) * 128,
                                            g * GROUP * N:(g + 1) * GROUP * N,
                                        ],
                                        ot[:],
                                    )
    nc.compile()
    return nc


def _prep_weights(w, b, bn_gamma, bn_beta, bn_mean, bn_var):
    inv = (bn_gamma / np.sqrt(bn_var + BN_EPS)).astype(np.float64)
    wf = (w[:, 0].astype(np.float64) * inv[:, None, None]).astype(np.float32)
    bf = (
        b.astype(np.float64) * inv + bn_beta.astype(np.float64)
        - bn_mean.astype(np.float64) * inv
    ).astype(np.float32)

    # wpack[k, cb, kw, ab, m]: block-diagonal lhsT matrices. Even blocks have
    # phase0 on k rows 0..63; odd blocks are mirrored (phase0 on 64..127).
    wpack = np.zeros((128, NBLK, KW, 2, 128), np.float32)
    m = np.arange(128)
    c = m // 2
    for cb in range(NBLK):
        o = 128 * cb + m
        p0, p1 = (c, 64 + c) if cb % 2 == 0 else (64 + c, c)
        for kw in range(KW):
            wpack[p0, cb, kw, 0, m] = wf[o, 0, kw]   # phase0 -> kh0
            wpack[p1, cb, kw, 0, m] = wf[o, 1, kw]   # phase1 -> kh1
            wpack[p1, cb, kw, 1, m] = wf[o, 2, kw]   # phase1 @+1row -> kh2
    wflat = wpack.reshape(128, NBLK * 6 * 128)
    bpack = np.ascontiguousarray(bf.reshape(NBLK, 128).T)
    return wflat, bpack


def _run(x, w, b, bn_gamma, bn_beta, bn_mean, bn_var, **run_kwargs):
    from concourse.bass_utils import run_bass_kernel_spmd

    if "nc" not in _state:
        _state["nc"] = _build_module()
    nc = _state["nc"]

    wflat, bpack = _prep_weights(w, b, bn_gamma, bn_beta, bn_mean, bn_var)
    xp = np.zeros((B, CIN, HP, WP), np.float32)
    xp[:, :, 1:H + 1, 1:W + 1] = x
    xp = xp.reshape(B, CIN, HP * WP)

    in_maps = [
        {"xp": xp[i * BPC:(i + 1) * BPC], "wpack": wflat, "bpack": bpack}
        for i in range(NCORES)
    ]
    res = run_bass_kernel_spmd(nc, in_maps, core_ids=list(range(NCORES)), **run_kwargs)
    y = np.concatenate([r["y"] for r in res.results], axis=0)
    return y.reshape(B, COUT, H, W), res


def kernel(x, w, b, bn_gamma, bn_beta, bn_mean, bn_var):
    x = np.ascontiguousarray(np.asarray(x, dtype=np.float32))
    w = np.asarray(w, dtype=np.float32)
    b = np.asarray(b, dtype=np.float32)
    bn_gamma = np.asarray(bn_gamma, dtype=np.float32)
    bn_beta = np.asarray(bn_beta, dtype=np.float32)
    bn_mean = np.asarray(bn_mean, dtype=np.float32)
    bn_var = np.asarray(bn_var, dtype=np.float32)
    y, _ = _run(x, w, b, bn_gamma, bn_beta, bn_mean, bn_var)
    return y
